# revision 10
# baseline (speedup 1.0000x reference)
"""LowRankSparse2to4Linear Trainium2 kernel (v3).

out = (x16 @ A16) -> fp16 -> (@ B16^T) + bias, where A16/B16 are the 2:4
soft-thresholded (along rank), scaled, fp16-cast low-rank factors.

Strategy (8 NeuronCores, data-parallel over tokens, NO collectives):
  - tokens (8192) sharded 1024/core; every core receives the FULL weights
    and redundantly preprocesses them on-chip.
  - Weight preprocessing works on PAIRS of 128-row chunks ([128, 2048]):
      * Act engine: one strided copy per pair produces V = fp16 cast of
        the weights in the deinterleaved (lane-split) rank layout.  The
        permutation is consistent between A and B^T so it cancels in the
        second GEMM's contraction.
      * DVE: M = abs_max(V,V); min/max tournament (P,Q,E,F,t) on packed
        fp16 at the DVE 2x rate; fused custom SOFT_SHRINK op applies
        sp = v - clamp(v,-t,t) from packed fp16 (1.3us/chunk measured).
  - soft(s*g) == s*soft(g) exactly, so scale_A/scale_B fold into the
    output stage.
  - ALL transposes run on the DMA XBAR (dma_start_transpose, sync queue
    only - the Act DGE returns wrong data), not the tensor engine:
    x is cast to fp16 then block-transposed into [128, col, tc, tok]
    slotted tiles; B^T likewise per weight chunk.  The tensor engine
    runs nothing but the two GEMM streams (~221us of fp16 matmul).
  - GEMM1 token-half 0 accumulates both rank-halves per contraction
    chunk (8 PSUM banks, 8 matmuls per arriving weight chunk) so the
    tensor engine tracks the preprocessing pipeline with minimal idle;
    half 1 runs at full speed from 4 banks while GEMM2 starts on the
    other 4.
"""

import os
import sys
import numpy as np

sys.path.insert(0, "/opt/trn_rl_repo")

N_CORES = 8
IN_F, OUT_F, RANK = 4096, 4096, 1024
T_FULL = 8192             # 4 * 2048 tokens
TPC = T_FULL // N_CORES   # 1024 tokens per core

_BUILD_CACHE = {}

_DVE_OPS = {}

# out-copy rotation: fraction of output copies on Act vs DVE (i % 8 < OUT_ACT)
OUT_ACT_OF_8 = 4


def _register_custom_dve_ops():
    """Register the fused SOFT_SHRINK DVE op:
    out = in0 - clamp(in0, -in1, in1)   (in1 >= 0)."""
    if _DVE_OPS:
        return _DVE_OPS
    import numpy as _np
    from concourse import dve_ops
    from concourse.dve_spec import (Spec, Src0, Src1, Zero, minn, maxx,
                                    select, lower, _has_src1)
    from concourse.dve_uop import DveOpSpec

    def make_op(name, body, ref):
        existing = {op.name: op for op in dve_ops.OPS}
        if name in existing:
            return existing[name]
        spec = Spec(body=body, reference=ref)
        row = dve_ops._CUSTOM_DVE_ROW_BASE + len(dve_ops.OPS)
        shas = {}
        for ver in ("v3", "v4"):
            try:
                tmp = DveOpSpec(name=name, opcode=row, uops=lower(spec, ver=ver),
                                rd1_en=_has_src1(spec))
                shas[ver] = tmp.sha(ver)
            except Exception:
                pass
        op = dve_ops.DveOp(name, spec, subdim=False, uops_sha=shas)
        dve_ops.OPS.append(op)
        dve_ops.CUSTOM_DVE_SPECS[name] = spec
        dve_ops._SUB_OPCODE_FOR_NAME[name] = row
        return op

    _DVE_OPS["shrink"] = make_op(
        "SOFT_SHRINK_ANT",
        select(Src0 < Zero, minn(Src0 + Src1, Zero), maxx(Src0 - Src1, Zero)),
        lambda in0, in1, s0, s1, imm2: _np.where(
            in0 < 0, _np.minimum(in0 + in1, 0), _np.maximum(in0 - in1, 0)))
    return _DVE_OPS


def _build(out_scale: float, bias_zero: bool):
    import concourse.bacc as bacc
    import concourse.tile as tile
    from concourse import mybir

    ops = _register_custom_dve_ops()

    f32 = mybir.dt.float32
    f16 = mybir.dt.float16
    Alu = mybir.AluOpType
    AF = mybir.ActivationFunctionType

    nc = bacc.Bacc("TRN2", target_bir_lowering=False, debug=False,
                   num_devices=N_CORES)

    x_sh = nc.dram_tensor("x_sh", [TPC, IN_F], f32, kind="ExternalInput")
    wa_d = nc.dram_tensor("wa_d", [IN_F, RANK], f32, kind="ExternalInput")
    wb_d = nc.dram_tensor("wb_d", [OUT_F, RANK], f32, kind="ExternalInput")
    bias_d = nc.dram_tensor("bias_d", [1, OUT_F], f32, kind="ExternalInput")
    out_d = nc.dram_tensor("out_d", [TPC, OUT_F], f32, kind="ExternalOutput")

    K_IN = IN_F // 128    # 32 contraction chunks for GEMM1
    K_RK = RANK // 128    # 8 contraction chunks for GEMM2
    N_TOK = TPC // 128    # 8 token chunks per core

    with tile.TileContext(nc) as tc:
        with (
            tc.tile_pool(name="singles", bufs=1) as singles,
            tc.tile_pool(name="st", bufs=2) as p_st,
            tc.tile_pool(name="vt", bufs=2) as p_v,
            tc.tile_pool(name="mtile", bufs=2) as p_m,
            tc.tile_pool(name="pq", bufs=2) as p_pq,
            tc.tile_pool(name="eft", bufs=3) as p_eft,
            tc.tile_pool(name="wasp", bufs=16) as p_wasp,
            tc.tile_pool(name="wbsp", bufs=4) as p_wbsp,
            tc.tile_pool(name="xf", bufs=3) as p_xf,
            tc.tile_pool(name="x16", bufs=3) as p_x16,
            tc.tile_pool(name="xt", bufs=4) as p_xt,
            tc.tile_pool(name="xproj", bufs=16) as p_xp,
            tc.tile_pool(name="wbt", bufs=2) as p_wbt,
            tc.tile_pool(name="oev", bufs=3) as p_out,
            tc.tile_pool(name="psg1", bufs=4, space="PSUM") as p_psg1,
            tc.tile_pool(name="psg2", bufs=4, space="PSUM") as p_psg2,
        ):
            if not bias_zero:
                bias_bc = singles.tile([128, OUT_F], f32)
                nc.sync.dma_start(bias_bc[0:1, :], bias_d[:])
                k = 1
                while k < 128:
                    nc.sync.dma_start(bias_bc[k:2 * k, :], bias_bc[0:k, :])
                    k *= 2

            def prep_pair(src_dram, pi, dst_pool, name):
                """Soft-threshold a PAIR of (128, RANK) f32 row chunks into
                one fp16 [128, 2*RANK] tile in the deinterleaved rank
                layout: out[:, 1024c + 256f + q] = soft(w)[:, 4q + f]."""
                st = p_st.tile([128, 2 * RANK], f32, tag="st",
                               name=f"st_{name}")
                r0 = pi * 256
                nc.sync.dma_start(st[:, 0:RANK], src_dram[r0:r0 + 128, :])
                nc.sync.dma_start(st[:, RANK:2 * RANK],
                                  src_dram[r0 + 128:r0 + 256, :])

                # V = fp16 deinterleaved values (one strided Act copy)
                V = p_v.tile([128, 2 * RANK], f16, tag="v", name=f"V_{name}")
                nc.scalar.copy(
                    V[:].rearrange("p (c f q) -> p c f q", c=2, f=4, q=256),
                    st[:].rearrange("p (c q f) -> p c f q", c=2, q=256, f=4))

                # M = |V|: clear the fp16 sign bit (packed u16, 2x DVE rate)
                M = p_m.tile([128, 2 * RANK], f16, tag="m", name=f"M_{name}")
                u16 = mybir.dt.uint16
                nc.vector.tensor_scalar(out=M[:].bitcast(u16),
                                        in0=V[:].bitcast(u16),
                                        scalar1=0x7FFF, scalar2=None,
                                        op0=Alu.bitwise_and)

                Mr = M[:].rearrange("p (c e) -> p c e", c=2)
                P = p_pq.tile([128, RANK], f16, tag="pq", name=f"P_{name}")
                Q = p_pq.tile([128, RANK], f16, tag="pq", name=f"Q_{name}")
                Pr = P[:].rearrange("p (c e) -> p c e", c=2)
                Qr = Q[:].rearrange("p (c e) -> p c e", c=2)
                nc.vector.tensor_tensor(out=Pr, in0=Mr[:, :, 0:512],
                                        in1=Mr[:, :, 512:1024], op=Alu.min)
                nc.vector.tensor_tensor(out=Qr, in0=Mr[:, :, 0:512],
                                        in1=Mr[:, :, 512:1024], op=Alu.max)
                E = p_eft.tile([128, 512], f16, tag="eft", name=f"E_{name}")
                F = p_eft.tile([128, 512], f16, tag="eft", name=f"F_{name}")
                T = p_eft.tile([128, 512], f16, tag="eft", name=f"T_{name}")
                Er = E[:].rearrange("p (c e) -> p c e", c=2)
                Fr = F[:].rearrange("p (c e) -> p c e", c=2)
                nc.vector.tensor_tensor(out=Er, in0=Pr[:, :, 0:256],
                                        in1=Pr[:, :, 256:512], op=Alu.max)
                nc.vector.tensor_tensor(out=Fr, in0=Qr[:, :, 0:256],
                                        in1=Qr[:, :, 256:512], op=Alu.min)
                nc.vector.tensor_tensor(out=T[:], in0=E[:], in1=F[:],
                                        op=Alu.min)

                wsp = dst_pool.tile([128, 2 * RANK], f16, tag="wsp",
                                    name=f"wsp_{name}")
                for c in range(2):
                    nc.vector._custom_dve(
                        ops["shrink"],
                        out=wsp[:, c * RANK:(c + 1) * RANK].rearrange(
                            "p (f q) -> p f q", f=4),
                        in0=V[:, c * RANK:(c + 1) * RANK].rearrange(
                            "p (f q) -> p f q", f=4),
                        in1=T[:, c * 256:(c + 1) * 256][:, None, :]
                            .to_broadcast([128, 4, 256]))
                return wsp

            # ---- x pipeline: cast + DMA-XBAR transpose, one tile at a time ----
            def x_tile(th, blk, tc4, xt4):
                tok0 = (th * 4 + tc4) * 128
                xf = p_xf.tile([128, 1024], f32, tag="xf",
                               name=f"xf_{th}_{blk}_{tc4}")
                nc.sync.dma_start(
                    xf[:], x_sh[tok0:tok0 + 128,
                                blk * 1024:(blk + 1) * 1024])
                x16 = p_x16.tile([128, 1024], f16, tag="x16",
                                 name=f"x16_{th}_{blk}_{tc4}")
                nc.scalar.copy(x16[:], xf[:])
                nc.sync.dma_start_transpose(xt4[:, :, tc4, :], x16[:])

            def x_blk(th, blk):
                """[128 in-sub, 8 col, 4 tc4, 128 tok] for one 1024-in blk."""
                xt = p_xt.tile([128, 4096], f16, tag="xt",
                               name=f"xT_{th}_{blk}")
                xt4 = xt[:].rearrange("p (c t f) -> p c t f", c=8, t=4, f=128)
                for tc4 in range(4):
                    x_tile(th, blk, tc4, xt4)
                return xt

            def gemm1_th0(xts, wa_pairs, xproj):
                """8-bank interleaved: both rank-halves accumulate per ic."""
                accs = ([p_psg1.tile([128, 512], f32, tag="g1",
                                     name=f"g1_0_0_{m}") for m in range(4)] +
                        [p_psg2.tile([128, 512], f32, tag="g2",
                                     name=f"g1_0_1_{m}") for m in range(4)])
                for ic in range(K_IN):
                    wa = wa_pairs[ic // 2]
                    base = (ic % 2) * RANK
                    xt4 = xts[ic // 8][:].rearrange(
                        "p (c t f) -> p c (t f)", c=8, t=4)
                    rhs = xt4[:, ic % 8, :]
                    for j in range(8):
                        nc.tensor.matmul(
                            accs[j][:],
                            wa[:, base + j * 128:base + (j + 1) * 128],
                            rhs,
                            start=(ic == 0), stop=(ic == K_IN - 1))
                for j in range(8):
                    xp = p_xp.tile([128, 512], f16, tag="xp",
                                   name=f"xp_0_{j}")
                    nc.scalar.copy(xp[:], accs[j][:])
                    xproj[(0, j)] = xp

            def gemm1_th1(xts, wa_pairs, xproj):
                """4-bank, full speed (all chunks ready)."""
                for mh in range(2):
                    accs = [p_psg1.tile([128, 512], f32, tag="g1",
                                        name=f"g1_1_{mh}_{m}")
                            for m in range(4)]
                    for ic in range(K_IN):
                        wa = wa_pairs[ic // 2]
                        base = (ic % 2) * RANK
                        xt4 = xts[ic // 8][:].rearrange(
                            "p (c t f) -> p c (t f)", c=8, t=4)
                        rhs = xt4[:, ic % 8, :]
                        for m in range(4):
                            nc.tensor.matmul(
                                accs[m][:],
                                wa[:, base + (mh * 4 + m) * 128:
                                   base + (mh * 4 + m + 1) * 128],
                                rhs,
                                start=(ic == 0), stop=(ic == K_IN - 1))
                    for m in range(4):
                        xp = p_xp.tile([128, 512], f16, tag="xp",
                                       name=f"xp_1_{mh}_{m}")
                        nc.scalar.copy(xp[:], accs[m][:])
                        xproj[(1, mh * 4 + m)] = xp

            # ================= emission =================
            # Interleave A-pair prep with the x-th0 pipeline so neither the
            # Act queue nor the DMA queues serialize the other: GEMM1-th0
            # consumes pair p at ~3.5us cadence and x-blk b by pair 4b.
            xts0 = []
            wa_pairs = []
            for g in range(4):
                xts0.append(x_blk(0, g))
                for pi in range(4 * g, 4 * g + 4):
                    wa_pairs.append(prep_pair(wa_d, pi, p_wasp, f"a{pi}"))

            xproj = {}
            gemm1_th0(xts0, wa_pairs, xproj)

            # x-th1 + early B pairs interleaved (runs while th0 computes)
            wb_pairs = {}
            xts1 = []
            for g in range(4):
                xts1.append(x_blk(1, g))
                if g < 3:
                    wb_pairs[g] = prep_pair(wb_d, g, p_wbsp, f"b{g}")

            gemm1_th1(xts1, wa_pairs, xproj)

            def copy_out(i, ot, acc, nb):
                if bias_zero:
                    if out_scale == 1.0:
                        if i % 8 < OUT_ACT_OF_8:
                            nc.scalar.copy(ot[:], acc[:])
                        else:
                            nc.vector.tensor_copy(out=ot[:], in_=acc[:])
                    else:
                        if i % 8 < OUT_ACT_OF_8:
                            nc.scalar.activation(ot[:], acc[:], AF.Copy,
                                                 scale=float(out_scale))
                        else:
                            nc.vector.tensor_scalar_mul(ot[:], acc[:],
                                                        float(out_scale))
                else:
                    if out_scale != 1.0:
                        nc.vector.tensor_scalar(out=acc[:], in0=acc[:],
                                                scalar1=float(out_scale),
                                                scalar2=None, op0=Alu.mult)
                    nc.vector.tensor_tensor(
                        out=ot[:], in0=acc[:],
                        in1=bias_bc[:, nb * 512:(nb + 1) * 512], op=Alu.add)

            oi = 0
            for nb in range(OUT_F // 512):
                for pi in (2 * nb, 2 * nb + 1):
                    if pi not in wb_pairs:
                        wb_pairs[pi] = prep_pair(wb_d, pi, p_wbsp, f"b{pi}")
                wsp_b = [wb_pairs.pop(2 * nb), wb_pairs.pop(2 * nb + 1)]
                wbt = p_wbt.tile([128, 4096], f16, tag="wbt",
                                 name=f"wbt_{nb}")
                wbt4 = wbt[:].rearrange("p (r w f) -> p r w f",
                                        r=8, w=4, f=128)
                for wc in range(4):
                    nc.sync.dma_start_transpose(
                        wbt4[:, :, wc, :],
                        wsp_b[wc // 2][:, (wc % 2) * RANK:
                                       (wc % 2 + 1) * RANK])
                # prefetch next nb's pairs so B-prep stays a block ahead
                for pi in (2 * nb + 2, 2 * nb + 3):
                    if pi < K_RK * 2 and pi not in wb_pairs:
                        wb_pairs[pi] = prep_pair(wb_d, pi, p_wbsp, f"b{pi}")
                wbt3 = wbt[:].rearrange("p (r w f) -> p r (w f)", r=8, w=4)
                for mt in range(N_TOK):
                    acc2 = p_psg2.tile([128, 512], f32, tag="g2",
                                       name=f"g2_{nb}_{mt}")
                    th, ml = mt // 4, mt % 4
                    for kc in range(K_RK):
                        nc.tensor.matmul(
                            acc2[:],
                            xproj[(th, kc)][:, ml * 128:(ml + 1) * 128],
                            wbt3[:, kc, :],
                            start=(kc == 0), stop=(kc == K_RK - 1))
                    ot = p_out.tile([128, 512], f32, tag="oev",
                                    name=f"ot_{nb}_{mt}")
                    copy_out(oi, ot, acc2, nb)
                    oi += 1
                    # two half-width DMAs -> smaller per-queue chunks,
                    # shorter final drain
                    for hh in range(2):
                        nc.sync.dma_start(
                            out_d[mt * 128:(mt + 1) * 128,
                                  nb * 512 + hh * 256:nb * 512 + (hh + 1) * 256],
                            ot[:, hh * 256:(hh + 1) * 256])

    nc.compile()
    return nc


def kernel(x, weight_A, weight_B, bias, scale_A, scale_B):
    from concourse.bass_utils import run_bass_kernel_spmd

    x = np.ascontiguousarray(np.asarray(x, dtype=np.float32))
    weight_A = np.ascontiguousarray(np.asarray(weight_A, dtype=np.float32))
    weight_B = np.ascontiguousarray(np.asarray(weight_B, dtype=np.float32))
    bias = np.ascontiguousarray(np.asarray(bias, dtype=np.float32))
    sa = float(np.asarray(scale_A))
    sb = float(np.asarray(scale_B))
    bias_zero = bool(np.all(bias == 0.0))

    lead = x.shape[:-1]
    xf = x.reshape(-1, IN_F)
    assert xf.shape == (T_FULL, IN_F)

    key = (sa * sb, bias_zero)
    if key not in _BUILD_CACHE:
        _BUILD_CACHE[key] = _build(sa * sb, bias_zero)
    nc = _BUILD_CACHE[key]

    bias_row = bias.reshape(1, OUT_F)
    in_maps = []
    for c in range(N_CORES):
        in_maps.append({
            "x_sh": xf[c * TPC:(c + 1) * TPC],
            "wa_d": weight_A,
            "wb_d": weight_B,
            "bias_d": bias_row,
        })

    trace = os.environ.get("BASS_KERNEL_TRACE", "0") == "1"
    kwargs = {}
    if trace:
        _install_ntff_hook()
        kwargs["trace"] = True
        tmpdir = os.environ.get("BASS_KERNEL_TRACE_DIR")
        if tmpdir:
            os.makedirs(tmpdir, exist_ok=True)
            kwargs["tmpdir"] = tmpdir

    res = run_bass_kernel_spmd(nc, in_maps, core_ids=list(range(N_CORES)),
                               **kwargs)
    if trace:
        kernel.last_exec_time_ns = res.exec_time_ns

    out = np.empty((T_FULL, OUT_F), dtype=np.float32)
    for c in range(N_CORES):
        out[c * TPC:(c + 1) * TPC] = res.results[c]["out_d"]
    return out.reshape(*lead, OUT_F)


def _install_ntff_hook():
    """Provide antenv.axon_hooks (missing in this image) so trace=True works."""
    import types
    if "antenv.axon_hooks" in sys.modules:
        return
    try:
        from trn_agent_boot.trn_boot import _ntff_profile_via_ctypes
        hook = _ntff_profile_via_ctypes("/opt/axon/libaxon_pjrt.so")
    except Exception:
        hook = None
    mod = types.ModuleType("antenv.axon_hooks")
    mod.get_axon_ntff_profile_hook = lambda: hook
    mod.set_axon_ntff_profile_hook = lambda h: None
    import antenv  # noqa: F401
    sys.modules["antenv.axon_hooks"] = mod
